# revision 2
# baseline (speedup 1.0000x reference)
"""ConcatRelationModule Bass kernel for 8 trn2 NeuronCores — v2.

Reference computation (per edge e in [0, 16383)):
    x      = concat(inputs[heads[e], 0, :], inputs[e + 1, 1, :])     # [512]
    h      = tanh(concat(x @ W_FOH, x @ W_FOM) + b1)                 # [1024]
    h2     = tanh(h @ W2 + b2)                                       # [256]
    out[e] = h2 @ W3 + b3                                            # [E, 64]

Strategy vs v1 (74.6us):
  - modifier (bwd) half of x is pre-transposed on the host and lands
    feature-major via one direct DMA -> no PE transposes for it
  - gathered head half is transposed with the DMA XBAR transpose
    (dma_start_transpose, 14ns per 16x128 tile) -> zero PE transposes
  - PE warm-up matmuls on scratch data during the DMA prologue so the
    HAM clock gate (1.2 -> 2.4 GHz after ~3.4us busy) flips before real
    matmuls start
  - three dense phases (L1, L2, L3) with no PE gaps; within L1 the
    modifier half of each group is accumulated first so the PE never
    waits on the gather
  - b3 is added on the host (outputs returned pre-bias from device)
Engine assignment: PE = matmuls only; Scalar = headsT load + tanh ACTs;
Sync = all other DMAs + XBAR transposes + output stores; GpSimd =
memset + indirect gathers (SWDGE is gpsimd-only); Vector = PSUM->SBUF
copies of the final [64, 512] tiles.
"""

import os

import numpy as np
import ml_dtypes

import concourse.bass as bass
import concourse.bacc as bacc
import concourse.mybir as mybir
import concourse.tile as tile
from concourse.bass import IndirectOffsetOnAxis
from concourse.bass_utils import run_bass_kernel_spmd

N_TOKENS = 16384
LD = 256          # ldims
HID = 512
HID2 = 256
NREL = 64
NCORES = 8
E = N_TOKENS - 1  # 16383 real edges
EPC = N_TOKENS // NCORES  # 2048 edges per core (last edge padded)
P = 128
GS = 512                  # edges per group (matmul free dim)
NG = EPC // GS            # 4 groups
SUB = EPC // P            # 16 gather subtiles of 128 edges
N_WARMUP = 10             # PE warm-up matmuls (N=512, ~4.3us cold)

LAST_RESULTS = None
_CACHE = {}


def _build():
    bf16 = mybir.dt.bfloat16
    f32 = mybir.dt.float32

    nc = bacc.Bacc()
    fwd = nc.declare_dram_parameter("fwd", [N_TOKENS, LD], bf16, isOutput=False)
    bwdT = nc.declare_dram_parameter("bwdT", [P, 2, EPC], bf16, isOutput=False)
    headsT = nc.declare_dram_parameter(
        "headsT", [P, SUB], mybir.dt.int32, isOutput=False)
    w1 = nc.declare_dram_parameter("w1", [2 * LD, 2 * HID], bf16, isOutput=False)
    w2p = nc.declare_dram_parameter("w2p", [P, 8, HID2], bf16, isOutput=False)
    w3p = nc.declare_dram_parameter("w3p", [P, 2, NREL], bf16, isOutput=False)
    bpack = nc.declare_dram_parameter("bpack", [P, 10], f32, isOutput=False)
    outT = nc.declare_dram_parameter("outT", [NREL, EPC], f32, isOutput=True)

    Tanh = mybir.ActivationFunctionType.Tanh

    with tile.TileContext(nc) as tc:
        with (
            tc.tile_pool(name="const", bufs=1) as const_pool,
            tc.tile_pool(name="xh", bufs=SUB) as xh_pool,
            tc.tile_pool(name="xT", bufs=NG) as xT_pool,
            tc.tile_pool(name="h1", bufs=NG) as h1_pool,
            tc.tile_pool(name="h2", bufs=NG) as h2_pool,
            tc.tile_pool(name="o", bufs=2) as o_pool,
            tc.tile_pool(name="ps", bufs=8, space="PSUM") as ps_pool,
        ):
            # -- head indices on the Scalar queue (Sync is busy with loads) --
            hT_sb = const_pool.tile([P, SUB], mybir.dt.int32)
            nc.scalar.dma_start(hT_sb[:], headsT[:])

            # -- gathers: SWDGE, gpsimd only.  One 128-row gather per subtile
            warm_sb = const_pool.tile([P, GS], bf16)
            nc.gpsimd.memset(warm_sb[:], 0)
            xh = []
            for s in range(SUB):
                t = xh_pool.tile([P, LD], bf16, tag="xh", name=f"xh_{s}")
                nc.gpsimd.indirect_dma_start(
                    out=t[:],
                    out_offset=None,
                    in_=fwd[:],
                    in_offset=IndirectOffsetOnAxis(ap=hT_sb[:, s:s + 1], axis=0),
                )
                xh.append(t)

            # -- bulk loads on Sync, in the order the PE will need them --
            bwdT_sb = const_pool.tile([P, 2, EPC], bf16)
            nc.sync.dma_start(bwdT_sb[:], bwdT[:])
            w1_sb = [const_pool.tile([P, 2 * HID], bf16, name=f"w1_{kc}")
                     for kc in range(4)]
            for kc in (2, 3):  # modifier-half chunks first
                nc.sync.dma_start(w1_sb[kc][:], w1[kc * P:(kc + 1) * P, :])
            bp_sb = const_pool.tile([P, 10], f32)
            nc.sync.dma_start(bp_sb[:], bpack[:])
            for kc in (0, 1):
                nc.sync.dma_start(w1_sb[kc][:], w1[kc * P:(kc + 1) * P, :])
            w2_sb = const_pool.tile([P, 8, HID2], bf16)
            nc.sync.dma_start(w2_sb[:], w2p[:])
            w3_sb = const_pool.tile([P, 2, NREL], bf16)
            nc.sync.dma_start(w3_sb[:], w3p[:])

            # -- XBAR transposes (Sync program order: after the loads above,
            #    before the output stores emitted in phase C).  Each converts
            #    a gathered [128 edge, 256 feat] subtile into feature-major
            #    [feat%128, feat//128, edge] slots of the group tile.
            xT = []
            for g in range(NG):
                tg = xT_pool.tile([P, 2, NG, P], bf16, tag="xT", name=f"xT_{g}")
                for sl in range(4):
                    nc.sync.dma_start_transpose(
                        out=tg[:, :, sl, :], in_=xh[g * 4 + sl][:])
                xT.append(tg)

            # -- PE warm-up: garbage matmuls to flip the HAM clock gate while
            #    the prologue DMAs are in flight.  Output is never read.
            wps = ps_pool.tile([P, GS], f32, tag="ps", name="warmup")
            for i in range(N_WARMUP):
                nc.tensor.matmul(
                    out=wps[:], lhsT=warm_sb[:, 0:P], rhs=warm_sb[:],
                    start=True, stop=True,
                )

            # ---- phase A: L1  h = tanh(x @ W1 + b1), feature-major ----
            h1 = []
            for g in range(NG):
                gs = slice(g * GS, (g + 1) * GS)
                ph = [ps_pool.tile([P, GS], f32, tag="ps", name=f"ph_{g}_{hc}")
                      for hc in range(8)]
                # modifier half first: no gather dependency
                for hc in range(8):
                    for i, kc in enumerate((2, 3)):
                        nc.tensor.matmul(
                            out=ph[hc][:],
                            lhsT=w1_sb[kc][:, hc * P:(hc + 1) * P],
                            rhs=bwdT_sb[:, kc - 2, gs],
                            start=(i == 0), stop=False,
                        )
                h1g = h1_pool.tile([P, 8, GS], bf16, tag="h1", name=f"h1_{g}")
                for hc in range(8):
                    for i, kc in enumerate((0, 1)):
                        nc.tensor.matmul(
                            out=ph[hc][:],
                            lhsT=w1_sb[kc][:, hc * P:(hc + 1) * P],
                            rhs=xT[g][:, kc, :, :],
                            start=False, stop=(i == 1),
                        )
                    nc.scalar.activation(
                        out=h1g[:, hc, :], in_=ph[hc][:], func=Tanh,
                        bias=bp_sb[:, hc:hc + 1],
                    )
                h1.append(h1g)

            # ---- phase B: L2  h2 = tanh(h @ W2 + b2) ----
            h2 = []
            for g in range(NG):
                h2g = h2_pool.tile([P, 2, GS], bf16, tag="h2", name=f"h2_{g}")
                for jc in range(2):
                    pj = ps_pool.tile([P, GS], f32, tag="ps", name=f"pj_{g}_{jc}")
                    for kc in range(8):
                        nc.tensor.matmul(
                            out=pj[:],
                            lhsT=w2_sb[:, kc, jc * P:(jc + 1) * P],
                            rhs=h1[g][:, kc, :],
                            start=(kc == 0), stop=(kc == 7),
                        )
                    nc.scalar.activation(
                        out=h2g[:, jc, :], in_=pj[:], func=Tanh,
                        bias=bp_sb[:, 8 + jc:9 + jc],
                    )
                h2.append(h2g)

            # ---- phase C: out = h2 @ W3  (b3 added on host) ----
            for g in range(NG):
                po = ps_pool.tile([NREL, GS], f32, tag="ps", name=f"po_{g}")
                for kc in range(2):
                    nc.tensor.matmul(
                        out=po[:],
                        lhsT=w3_sb[:, kc, :],
                        rhs=h2[g][:, kc, :],
                        start=(kc == 0), stop=(kc == 1),
                    )
                og = o_pool.tile([NREL, GS], f32, tag="o", name=f"o_{g}")
                nc.vector.tensor_copy(out=og[:], in_=po[:])
                nc.sync.dma_start(outT[:, g * GS:(g + 1) * GS], og[:])

    nc.finalize()
    return nc


def _prep_inputs(inputs, rhidLayerFOH, rhidLayerFOM, rcatBias, rhid2Layer,
                 rhid2Bias, routLayer, routBias, heads):
    """Host-side shard/pack.  Returns per-core input maps."""
    wdt = ml_dtypes.bfloat16
    inputs = np.asarray(inputs, dtype=np.float32)
    heads = np.asarray(heads)

    fwd = np.ascontiguousarray(inputs[:, 0, :]).astype(wdt)      # [N, 256]
    bwd_full = inputs[:, 1, :]                                   # [N, 256]
    # mod of edge e is e+1; pad edge 16383 with token 16383 (garbage, dropped)
    mods_pad = np.concatenate(
        [np.arange(1, N_TOKENS), [N_TOKENS - 1]]).astype(np.int64)
    heads_pad = np.concatenate([heads.astype(np.int64), [0]]).astype(np.int32)

    w1 = np.ascontiguousarray(
        np.concatenate([np.asarray(rhidLayerFOH), np.asarray(rhidLayerFOM)],
                       axis=1)).astype(wdt)                      # [512, 1024]
    w2p = np.ascontiguousarray(
        np.asarray(rhid2Layer, dtype=np.float32)
        .reshape(8, P, HID2)
        .transpose(1, 0, 2)).astype(wdt)                         # [128, 8, 256]
    w3p = np.ascontiguousarray(
        np.asarray(routLayer, dtype=np.float32)
        .reshape(2, P, NREL)
        .transpose(1, 0, 2)).astype(wdt)                         # [128, 2, 64]
    b1 = np.asarray(rcatBias, dtype=np.float32).reshape(8, P).T  # [128, 8]
    b2 = np.asarray(rhid2Bias, dtype=np.float32).reshape(2, P).T  # [128, 2]
    bpack = np.ascontiguousarray(
        np.concatenate([b1, b2], axis=1))                        # [128, 10]

    in_maps = []
    for c in range(NCORES):
        sl = slice(c * EPC, (c + 1) * EPC)
        bwd_c = bwd_full[mods_pad[sl]]                           # [2048, 256]
        bwdT_c = np.ascontiguousarray(
            bwd_c.T.reshape(2, P, EPC).transpose(1, 0, 2)).astype(wdt)
        headsT_c = np.ascontiguousarray(
            heads_pad[sl].reshape(SUB, P).T)                     # [128, 16]
        in_maps.append({
            "fwd": fwd, "bwdT": bwdT_c, "headsT": headsT_c,
            "w1": w1, "w2p": w2p, "w3p": w3p, "bpack": bpack,
        })
    return in_maps


def kernel(inputs, rhidLayerFOH, rhidLayerFOM, rcatBias, rhid2Layer, rhid2Bias,
           routLayer, routBias, heads):
    global LAST_RESULTS

    in_maps = _prep_inputs(inputs, rhidLayerFOH, rhidLayerFOM, rcatBias,
                           rhid2Layer, rhid2Bias, routLayer, routBias, heads)

    if "nc" not in _CACHE:
        _CACHE["nc"] = _build()
    nc = _CACHE["nc"]

    trace_dir = os.environ.get("KERNEL_TRACE_DIR") or None
    res = run_bass_kernel_spmd(nc, in_maps, list(range(NCORES)), tmpdir=trace_dir)
    LAST_RESULTS = res

    outT = np.concatenate([r["outT"] for r in res.results], axis=1)  # [64, 16384]
    out = outT.T[:E] + np.asarray(routBias, dtype=np.float32)[None, :]
    return np.ascontiguousarray(out).astype(np.float32)


# revision 3
# speedup vs baseline: 1.1337x; 1.1337x over previous
"""ConcatRelationModule Bass kernel for 8 trn2 NeuronCores — v3.

Per edge e in [0, 16383):
    x      = concat(inputs[heads[e], 0, :], inputs[e + 1, 1, :])     # [512]
    h      = tanh(concat(x @ W_FOH, x @ W_FOM) + b1)                 # [1024]
    h2     = tanh(h @ W2 + b2)                                       # [256]
    out[e] = h2 @ W3 + b3                                            # [E, 64]

v3 structure (per core, 2048 edges in 4 groups of 512):
  - modifier (bwd) half of x host-pretransposed, one direct DMA
  - head half gathered on-device (SWDGE, 16x 128-row indirect DMAs),
    then transposed with ONE DMA XBAR transpose per 512-edge group
    ([128e,1024] -> m-major [128,8,128]); matmul reads it with a
    strided 3D rhs AP.  Zero PE transposes.
  - PE warm-up matmuls on scratch during the prologue (HAM clock gate)
  - phase-separated L1 / L2 / L3; within each L1 group the modifier
    half accumulates first so the PE never waits on the gather
  - b3 added on the host; output tiles copied PSUM->SBUF on Vector
    (groups 0,2) and Scalar ACT-identity (1,3), stored from both Sync
    and Scalar queues to shorten the tail
"""

import os

import numpy as np
import ml_dtypes

import concourse.bass as bass
import concourse.bacc as bacc
import concourse.mybir as mybir
import concourse.tile as tile
from concourse.bass import IndirectOffsetOnAxis
from concourse.bass_utils import run_bass_kernel_spmd

N_TOKENS = 16384
LD = 256
HID = 512
HID2 = 256
NREL = 64
NCORES = 8
E = N_TOKENS - 1
EPC = N_TOKENS // NCORES  # 2048
P = 128
GS = 512
NG = EPC // GS            # 4
SUB = EPC // P            # 16
N_WARMUP = 9

# transpose implementation: "xbar_group" | "xbar_sub" | "pe"
TMODE = os.environ.get("KERNEL_TMODE", "xbar_group")

LAST_RESULTS = None
_CACHE = {}


def _build(tmode):
    bf16 = mybir.dt.bfloat16
    f32 = mybir.dt.float32

    nc = bacc.Bacc()
    fwd = nc.declare_dram_parameter("fwd", [N_TOKENS, LD], bf16, isOutput=False)
    bwdT = nc.declare_dram_parameter("bwdT", [P, 2, EPC], bf16, isOutput=False)
    headsT = nc.declare_dram_parameter(
        "headsT", [P, SUB], mybir.dt.int32, isOutput=False)
    # w1 packed by k-chunk: w1a = modifier rows (256..511), w1b = head rows
    w1a = nc.declare_dram_parameter("w1a", [P, 2, 2 * HID], bf16, isOutput=False)
    w1b = nc.declare_dram_parameter("w1b", [P, 2, 2 * HID], bf16, isOutput=False)
    w2p = nc.declare_dram_parameter("w2p", [P, 8, HID2], bf16, isOutput=False)
    w3p = nc.declare_dram_parameter("w3p", [P, 2, NREL], bf16, isOutput=False)
    bpack = nc.declare_dram_parameter("bpack", [P, 10], f32, isOutput=False)
    outT = nc.declare_dram_parameter("outT", [NREL, EPC], f32, isOutput=True)

    Tanh = mybir.ActivationFunctionType.Tanh
    Ident = mybir.ActivationFunctionType.Identity

    if tmode == "pe":
        from concourse.masks import make_identity

    with tile.TileContext(nc) as tc:
        with (
            tc.tile_pool(name="const", bufs=1) as const_pool,
            tc.tile_pool(name="xh", bufs=NG) as xh_pool,
            tc.tile_pool(name="xT", bufs=NG) as xT_pool,
            tc.tile_pool(name="h1", bufs=NG) as h1_pool,
            tc.tile_pool(name="h2", bufs=NG) as h2_pool,
            tc.tile_pool(name="o", bufs=4) as o_pool,
            tc.tile_pool(name="ps", bufs=8, space="PSUM") as ps_pool,
        ):
            # headsT on the Scalar queue (Sync is loaded with bulk DMAs)
            hT_sb = const_pool.tile([P, SUB], mybir.dt.int32)
            nc.scalar.dma_start(hT_sb[:], headsT[:])

            # gathers: SWDGE, gpsimd-only; one 128-row gather per subtile,
            # 4 subtiles per group into one [128, 4, 256] tile
            warm_sb = const_pool.tile([P, GS], bf16)
            nc.gpsimd.memset(warm_sb[:], 0)
            xh = []
            for g in range(NG):
                t = xh_pool.tile([P, 4, LD], bf16, tag="xh", name=f"xh_{g}")
                for sl in range(4):
                    nc.gpsimd.indirect_dma_start(
                        out=t[:, sl, :],
                        out_offset=None,
                        in_=fwd[:],
                        in_offset=IndirectOffsetOnAxis(
                            ap=hT_sb[:, g * 4 + sl:g * 4 + sl + 1], axis=0),
                    )
                xh.append(t)

            # bulk loads on Sync in PE-need order
            bwdT_sb = const_pool.tile([P, 2, EPC], bf16)
            nc.sync.dma_start(bwdT_sb[:], bwdT[:])
            w1a_sb = const_pool.tile([P, 2, 2 * HID], bf16)
            nc.sync.dma_start(w1a_sb[:], w1a[:])
            bp_sb = const_pool.tile([P, 10], f32)
            nc.sync.dma_start(bp_sb[:], bpack[:])
            w1b_sb = const_pool.tile([P, 2, 2 * HID], bf16)
            nc.sync.dma_start(w1b_sb[:], w1b[:])
            w2_sb = const_pool.tile([P, 8, HID2], bf16)
            nc.sync.dma_start(w2_sb[:], w2p[:])
            w3_sb = const_pool.tile([P, 2, NREL], bf16)
            nc.sync.dma_start(w3_sb[:], w3p[:])

            if tmode == "pe":
                ident = const_pool.tile([P, P], bf16)
                make_identity(nc, ident[:])

            # transposes -> per-group feature-major tiles.
            # xbar_group: out[p, m, e] = xh[e, m*128+p], m = sl*2 + kc
            #   -> matmul rhs for chunk kc is the strided view [:, kc::2, :]
            xT = []

            def emit_transpose(g):
                if tmode == "xbar_group":
                    tg = xT_pool.tile([P, 8, P], bf16, tag="xT", name=f"xT_{g}")
                    nc.sync.dma_start_transpose(out=tg[:], in_=xh[g][:])
                elif tmode == "xbar_sub":
                    tg = xT_pool.tile([P, 8, P], bf16, tag="xT", name=f"xT_{g}")
                    for sl in range(4):
                        nc.sync.dma_start_transpose(
                            out=tg[:, 2 * sl:2 * sl + 2, :], in_=xh[g][:, sl, :])
                else:  # pe
                    tg = xT_pool.tile([P, 8, P], bf16, tag="xT", name=f"xT_{g}")
                    for kc in range(2):
                        pt = ps_pool.tile([P, GS], bf16, tag="ps",
                                          name=f"pt_{g}_{kc}")
                        for sl in range(4):
                            nc.tensor.transpose(
                                pt[:, sl * P:(sl + 1) * P],
                                xh[g][:, sl, kc * P:(kc + 1) * P], ident[:])
                        nc.vector.tensor_copy(
                            out=tg[:].rearrange("p (s k) e -> p k s e", k=2)[:, kc, :, :],
                            in_=pt[:])
                xT.append(tg)

            def head_rhs(g, kc):
                # [128, 4, 128] strided view: m = sl*2 + kc
                return xT[g][:].rearrange("p (s k) e -> p k s e", k=2)[:, kc, :, :]

            for g in range(NG):
                emit_transpose(g)

            # PE warm-up on scratch; output never read
            wps = ps_pool.tile([P, GS], f32, tag="ps", name="warmup")
            for i in range(N_WARMUP):
                nc.tensor.matmul(
                    out=wps[:], lhsT=warm_sb[:, 0:P], rhs=warm_sb[:],
                    start=True, stop=True,
                )

            # ---- phase A: L1 ----
            h1 = []
            for g in range(NG):
                gs = slice(g * GS, (g + 1) * GS)
                ph = [ps_pool.tile([P, GS], f32, tag="ps", name=f"ph_{g}_{hc}")
                      for hc in range(8)]
                for hc in range(8):  # modifier half first: no gather dep
                    for i in range(2):
                        nc.tensor.matmul(
                            out=ph[hc][:],
                            lhsT=w1a_sb[:, i, hc * P:(hc + 1) * P],
                            rhs=bwdT_sb[:, i, gs],
                            start=(i == 0), stop=False,
                        )
                h1g = h1_pool.tile([P, 8, GS], bf16, tag="h1", name=f"h1_{g}")
                for hc in range(8):
                    for i in range(2):
                        nc.tensor.matmul(
                            out=ph[hc][:],
                            lhsT=w1b_sb[:, i, hc * P:(hc + 1) * P],
                            rhs=head_rhs(g, i),
                            start=False, stop=(i == 1),
                        )
                    nc.scalar.activation(
                        out=h1g[:, hc, :], in_=ph[hc][:], func=Tanh,
                        bias=bp_sb[:, hc:hc + 1],
                    )
                h1.append(h1g)

            # ---- phase B: L2 ----
            h2 = []
            for g in range(NG):
                h2g = h2_pool.tile([P, 2, GS], bf16, tag="h2", name=f"h2_{g}")
                for jc in range(2):
                    pj = ps_pool.tile([P, GS], f32, tag="ps", name=f"pj_{g}_{jc}")
                    for kc in range(8):
                        nc.tensor.matmul(
                            out=pj[:],
                            lhsT=w2_sb[:, kc, jc * P:(jc + 1) * P],
                            rhs=h1[g][:, kc, :],
                            start=(kc == 0), stop=(kc == 7),
                        )
                    nc.scalar.activation(
                        out=h2g[:, jc, :], in_=pj[:], func=Tanh,
                        bias=bp_sb[:, 8 + jc:9 + jc],
                    )
                h2.append(h2g)

            # ---- phase C: out = h2 @ W3 (b3 on host) ----
            for g in range(NG):
                po = ps_pool.tile([NREL, GS], f32, tag="ps", name=f"po_{g}")
                for kc in range(2):
                    nc.tensor.matmul(
                        out=po[:],
                        lhsT=w3_sb[:, kc, :],
                        rhs=h2[g][:, kc, :],
                        start=(kc == 0), stop=(kc == 1),
                    )
                og = o_pool.tile([NREL, GS], f32, tag="o", name=f"o_{g}")
                if g % 2 == 0:
                    nc.vector.tensor_copy(out=og[:], in_=po[:])
                    nc.sync.dma_start(outT[:, g * GS:(g + 1) * GS], og[:])
                else:
                    nc.scalar.activation(out=og[:], in_=po[:], func=Ident)
                    nc.scalar.dma_start(outT[:, g * GS:(g + 1) * GS], og[:])

    nc.finalize()
    return nc


def _prep_inputs(inputs, rhidLayerFOH, rhidLayerFOM, rcatBias, rhid2Layer,
                 rhid2Bias, routLayer, routBias, heads):
    wdt = ml_dtypes.bfloat16
    inputs = np.asarray(inputs, dtype=np.float32)
    heads = np.asarray(heads)

    fwd = np.ascontiguousarray(inputs[:, 0, :]).astype(wdt)
    bwd_full = inputs[:, 1, :]
    mods_pad = np.concatenate(
        [np.arange(1, N_TOKENS), [N_TOKENS - 1]]).astype(np.int64)
    heads_pad = np.concatenate([heads.astype(np.int64), [0]]).astype(np.int32)

    w1 = np.concatenate(
        [np.asarray(rhidLayerFOH), np.asarray(rhidLayerFOM)],
        axis=1).astype(np.float32)                               # [512, 1024]
    w1a = np.ascontiguousarray(
        w1[2 * P:].reshape(2, P, 2 * HID).transpose(1, 0, 2)).astype(wdt)
    w1b = np.ascontiguousarray(
        w1[:2 * P].reshape(2, P, 2 * HID).transpose(1, 0, 2)).astype(wdt)
    w2p = np.ascontiguousarray(
        np.asarray(rhid2Layer, dtype=np.float32)
        .reshape(8, P, HID2).transpose(1, 0, 2)).astype(wdt)
    w3p = np.ascontiguousarray(
        np.asarray(routLayer, dtype=np.float32)
        .reshape(2, P, NREL).transpose(1, 0, 2)).astype(wdt)
    b1 = np.asarray(rcatBias, dtype=np.float32).reshape(8, P).T
    b2 = np.asarray(rhid2Bias, dtype=np.float32).reshape(2, P).T
    bpack = np.ascontiguousarray(np.concatenate([b1, b2], axis=1))

    in_maps = []
    for c in range(NCORES):
        sl = slice(c * EPC, (c + 1) * EPC)
        bwd_c = bwd_full[mods_pad[sl]]
        bwdT_c = np.ascontiguousarray(
            bwd_c.T.reshape(2, P, EPC).transpose(1, 0, 2)).astype(wdt)
        headsT_c = np.ascontiguousarray(heads_pad[sl].reshape(SUB, P).T)
        in_maps.append({
            "fwd": fwd, "bwdT": bwdT_c, "headsT": headsT_c,
            "w1a": w1a, "w1b": w1b, "w2p": w2p, "w3p": w3p, "bpack": bpack,
        })
    return in_maps


def kernel(inputs, rhidLayerFOH, rhidLayerFOM, rcatBias, rhid2Layer, rhid2Bias,
           routLayer, routBias, heads):
    global LAST_RESULTS

    in_maps = _prep_inputs(inputs, rhidLayerFOH, rhidLayerFOM, rcatBias,
                           rhid2Layer, rhid2Bias, routLayer, routBias, heads)

    if TMODE not in _CACHE:
        _CACHE[TMODE] = _build(TMODE)
    nc = _CACHE[TMODE]

    trace_dir = os.environ.get("KERNEL_TRACE_DIR") or None
    res = run_bass_kernel_spmd(nc, in_maps, list(range(NCORES)), tmpdir=trace_dir)
    LAST_RESULTS = res

    outT = np.concatenate([r["outT"] for r in res.results], axis=1)
    out = outT.T[:E] + np.asarray(routBias, dtype=np.float32)[None, :]
    return np.ascontiguousarray(out).astype(np.float32)
